# revision 23
# baseline (speedup 1.0000x reference)
"""Trainium2 Bass kernel for nn_DualInt8LinearConv.

Reference computation (N=8192, Cin=4096, Cout=4096):
    x2d      = x.reshape(N, Cin)
    amax     = max(|x2d|, axis=1)
    scale_x  = softplus(amax) / 32767
    xq       = round(x2d / scale_x)                    # int16-valued
    y        = (xq@w0.T * s0 + xq@w1.T * s1) * scale_x + bias

Key identity: s0/s1 are per-Cout scalars, so
    xq@w0.T * s0 + xq@w1.T * s1  ==  xq @ (s0[:,None]*w0 + s1[:,None]*w1).T
and (xq * scale_x) == x up to the int16 fake-quant rounding (|eps| <=
scale_x/2 ~ 6e-5, never clipped since softplus(amax) > amax). Hence

    y  ~=  x @ Wc.T + bias,   Wc = s0[:,None]*w0 + s1[:,None]*w1

exactly up to quantization noise. Computing this single GEMM with x and
Wc rounded to bf16 (fp32 PSUM accumulation) gives max scale-relative
error ~2.4e-3 on the reference data — an order of magnitude inside the
2e-2 gate — while doing 1/4 of the matmul work of the exact dual-plane
int16xint8 decomposition.

Strategy:
  * Row-shard N across 8 cores (1024 rows each); Wc replicated.
  * Host prep: Wc combined + cast bf16 + pre-tiled [n_og, 128, n_ct*128]
    so each 128-Cout group streams as contiguous DMAs; x transposed to
    xT [Cin, n_shard] bf16 (contraction on partitions, no on-device
    transposes anywhere).
  * Device: xT resident in SBUF (8 MB). For each Cout group og:
    stream w tile, accumulate psum[128, n_shard] over 32 Cin tiles,
    epilogue = single vector op (psum + bias -> SBUF), DMA out yT.
  * Output yT [Cout, n_shard] f32; host transposes back.
"""

import os
import sys

sys.path.insert(0, "/opt/trn_rl_repo")

from contextlib import ExitStack

import numpy as np
import ml_dtypes

import concourse.bass as bass
import concourse.mybir as mybir
from concourse import bacc
import concourse.tile as tile

F32 = mybir.dt.float32
BF16 = mybir.dt.bfloat16
AF = mybir.ActivationFunctionType
ALU = mybir.AluOpType

N_FULL, CIN, COUT = 8192, 4096, 4096
NCORES = 8


def build_nc(n_shard=N_FULL // NCORES, cin=CIN, cout=COUT):
    n_ct = cin // 128       # Cin tiles (contraction)
    n_og = cout // 128      # Cout groups
    HB = min(512, n_shard)  # matmul moving width (one PSUM bank, f32)
    NH = n_shard // HB
    QS = min(8, n_ct)       # cts per weight quarter-tile
    NQ = n_ct // QS         # weight DMAs per og

    nc = bacc.Bacc()
    # x as plain xT [cin, n_shard]: each ct DMA reads one fully
    # contiguous 256KB block (best HBM locality in the congested head)
    xtb = nc.declare_dram_parameter("xtb", [cin, n_shard], BF16, isOutput=False)
    wch = nc.declare_dram_parameter("wch", [n_og, 128, n_ct * 128], BF16, isOutput=False)
    bp = nc.declare_dram_parameter("bp", [128, n_og], F32, isOutput=False)
    yt = nc.declare_dram_parameter("yt", [cout, n_shard], F32, isOutput=True)

    with tile.TileContext(nc) as tc, ExitStack() as ctx:
        p_x = ctx.enter_context(tc.tile_pool(name="xres", bufs=1))
        p_w = ctx.enter_context(tc.tile_pool(name="wts", bufs=8))
        p_out = ctx.enter_context(tc.tile_pool(name="out", bufs=4))
        p_s = ctx.enter_context(tc.tile_pool(name="svec", bufs=1))

        ball = p_s.tile([128, n_og], F32)
        nc.sync.dma_start(ball, bp[:])

        # xT resident in SBUF (one big tile; slice-level deps let og0
        # consume ct tiles as they land; gpsimd queue, one engine =>
        # one monotone semaphore for all x waits)
        x_res = p_x.tile([128, n_ct, n_shard], BF16)
        for ct in range(n_ct):
            nc.gpsimd.dma_start(
                x_res[:, ct, :], xtb[ct * 128:(ct + 1) * 128, :]
            )

        with tc.tile_pool(name="ps", bufs=4, space="PSUM") as p_ps:
            for og in range(n_og):
                # stream this og's weights: NQ quarter tiles (sync queue)
                wq = []
                for q in range(NQ):
                    wt_ = p_w.tile([128, QS * 128], BF16, tag="w")
                    nc.sync.dma_start(
                        wt_, wch[og, :, q * QS * 128:(q + 1) * QS * 128]
                    )
                    wq.append(wt_)

                ps = p_ps.tile([128, n_shard], F32, tag="ps")
                for ct in range(n_ct):
                    lhs = wq[ct // QS][:, (ct % QS) * 128:(ct % QS + 1) * 128]
                    first, last = ct == 0, ct == n_ct - 1
                    for nb in range(NH):
                        ns = slice(nb * HB, (nb + 1) * HB)
                        nc.tensor.matmul(
                            ps[:, ns], lhs, x_res[:, ct, ns],
                            start=first, stop=last,
                        )

                # epilogue: y = psum + bias (per-Cout col), then store
                t1 = p_out.tile([128, n_shard], F32, tag="t1")
                nc.vector.tensor_scalar_add(t1, ps, ball[:, og:og + 1])
                nc.scalar.dma_start(yt[og * 128:(og + 1) * 128, :], t1)

    nc.finalize()
    return nc


def _prep_weights(w, n_og, n_ct):
    # [cout, cin] -> [n_og, 128(p=cin sub), n_ct*128(o)] where
    # out[og, p, ct*128+o] = w[og*128+o, ct*128+p]
    cout, cin = w.shape
    wr = w.reshape(n_og, 128, n_ct, 128)        # [og, o, ct, p]
    wr = wr.transpose(0, 3, 2, 1)               # [og, p, ct, o]
    return np.ascontiguousarray(wr.reshape(n_og, 128, n_ct * 128)).astype(
        ml_dtypes.bfloat16
    )


def kernel(x, w0, w1, s0, s1, bias):
    from concourse.bass_utils import run_bass_kernel_spmd

    N, Cin = x.shape[0], x.shape[1]
    Cout = w0.shape[0]
    n_shard = N // NCORES
    n_ct = Cin // 128
    n_og = Cout // 128

    x2d = np.asarray(x, dtype=np.float32).reshape(N, Cin)
    Wc = (
        np.asarray(s0, np.float32)[:, None] * np.asarray(w0, np.float32)
        + np.asarray(s1, np.float32)[:, None] * np.asarray(w1, np.float32)
    )
    wch = _prep_weights(Wc, n_og, n_ct)
    bpp = np.ascontiguousarray(
        np.asarray(bias, np.float32).reshape(n_og, 128).T
    )

    nc = build_nc(n_shard=n_shard, cin=Cin, cout=Cout)

    in_maps = []
    for k in range(NCORES):
        xtk = np.ascontiguousarray(
            x2d[k * n_shard:(k + 1) * n_shard].T
        ).astype(ml_dtypes.bfloat16)
        in_maps.append({"xtb": xtk, "wch": wch, "bp": bpp})

    res = run_bass_kernel_spmd(
        nc,
        in_maps,
        core_ids=list(range(NCORES)),
        trace=bool(int(os.environ.get("KERNEL_TRACE", "0"))),
    )

    y = np.empty((N, Cout), dtype=np.float32)
    for k in range(NCORES):
        y[k * n_shard:(k + 1) * n_shard] = res.results[k]["yt"].T
    out = y.reshape(N, Cout, 1, 1)
    kernel.last_results = res
    return out


# revision 27
# speedup vs baseline: 1.0050x; 1.0050x over previous
"""Trainium2 Bass kernel for nn_DualInt8LinearConv.

Reference computation (N=8192, Cin=4096, Cout=4096):
    x2d      = x.reshape(N, Cin)
    amax     = max(|x2d|, axis=1)
    scale_x  = softplus(amax) / 32767
    xq       = round(x2d / scale_x)                    # int16-valued
    y        = (xq@w0.T * s0 + xq@w1.T * s1) * scale_x + bias

Key identity: s0/s1 are per-Cout scalars, so
    xq@w0.T * s0 + xq@w1.T * s1  ==  xq @ (s0[:,None]*w0 + s1[:,None]*w1).T
and (xq * scale_x) == x up to the int16 fake-quant rounding (|eps| <=
scale_x/2 ~ 6e-5, never clipped since softplus(amax) > amax). Hence

    y  ~=  x @ Wc.T + bias,   Wc = s0[:,None]*w0 + s1[:,None]*w1

exactly up to quantization noise. Computing this single GEMM with x and
Wc rounded to bf16 (fp32 PSUM accumulation) gives max scale-relative
error ~2.4e-3 on the reference data — an order of magnitude inside the
2e-2 gate — while doing 1/4 of the matmul work of the exact dual-plane
int16xint8 decomposition.

Strategy:
  * Row-shard N across 8 cores (1024 rows each); Wc replicated.
  * Host prep: Wc combined + cast bf16 + pre-tiled [n_og, 128, n_ct*128]
    so each 128-Cout group streams as contiguous DMAs; x transposed to
    xT [Cin, n_shard] bf16 (contraction on partitions, no on-device
    transposes anywhere).
  * Device: xT resident in SBUF (8 MB). For each Cout group og:
    stream w tile, accumulate psum[128, n_shard] over 32 Cin tiles,
    epilogue = single vector op (psum + bias -> SBUF), DMA out yT.
  * Output yT [Cout, n_shard] f32; host transposes back.
"""

import os
import sys

sys.path.insert(0, "/opt/trn_rl_repo")

from contextlib import ExitStack

import numpy as np
import ml_dtypes

import concourse.bass as bass
import concourse.mybir as mybir
from concourse import bacc
import concourse.tile as tile

F32 = mybir.dt.float32
BF16 = mybir.dt.bfloat16
AF = mybir.ActivationFunctionType
ALU = mybir.AluOpType

N_FULL, CIN, COUT = 8192, 4096, 4096
NCORES = 8


def build_nc(n_shard=N_FULL // NCORES, cin=CIN, cout=COUT):
    n_ct = cin // 128       # Cin tiles (contraction)
    n_og = cout // 128      # Cout groups
    HB = min(512, n_shard)  # matmul moving width (one PSUM bank, f32)
    NH = n_shard // HB
    QS = min(8, n_ct)       # cts per weight quarter-tile
    NQ = n_ct // QS         # weight DMAs per og

    nc = bacc.Bacc()
    # x host-tiled as [n_ct/2, 128, 2, n_shard]: chunk k holds ct pair
    # (2k, 2k+1) partition-major, so each 512KB chunk DMA is fully
    # contiguous in DRAM and lands partition-contiguous in SBUF.
    xtb = nc.declare_dram_parameter(
        "xtb", [n_ct // 2, 128, 2, n_shard], BF16, isOutput=False
    )
    wch = nc.declare_dram_parameter("wch", [n_og, 128, n_ct * 128], BF16, isOutput=False)
    bp = nc.declare_dram_parameter("bp", [128, n_og], F32, isOutput=False)
    yt = nc.declare_dram_parameter("yt", [cout, n_shard], F32, isOutput=True)

    with tile.TileContext(nc) as tc, ExitStack() as ctx:
        p_x = ctx.enter_context(tc.tile_pool(name="xres", bufs=1))
        p_w = ctx.enter_context(tc.tile_pool(name="wts", bufs=8))
        p_out = ctx.enter_context(tc.tile_pool(name="out", bufs=4))
        p_s = ctx.enter_context(tc.tile_pool(name="svec", bufs=1))

        ball = p_s.tile([128, n_og], F32)
        nc.sync.dma_start(ball, bp[:])

        # xT resident in SBUF (one big tile; slice-level deps let og0
        # consume ct tiles as they land; gpsimd queue, one engine =>
        # one monotone semaphore for all x waits). 2-ct chunks: issue
        # pace 0.65us/2ct beats og0's 0.88us/2ct consumption, and the
        # first 512KB chunk lands ~1.4us after issue.
        x_res = p_x.tile([128, n_ct, n_shard], BF16)
        for k in range(n_ct // 2):
            nc.gpsimd.dma_start(x_res[:, 2 * k:2 * k + 2, :], xtb[k])

        with tc.tile_pool(name="ps", bufs=4, space="PSUM") as p_ps:
            for og in range(n_og):
                # stream this og's weights: NQ quarter tiles (sync queue)
                wq = []
                for q in range(NQ):
                    wt_ = p_w.tile([128, QS * 128], BF16, tag="w")
                    nc.sync.dma_start(
                        wt_, wch[og, :, q * QS * 128:(q + 1) * QS * 128]
                    )
                    wq.append(wt_)

                ps = p_ps.tile([128, n_shard], F32, tag="ps")
                for ct in range(n_ct):
                    lhs = wq[ct // QS][:, (ct % QS) * 128:(ct % QS + 1) * 128]
                    first, last = ct == 0, ct == n_ct - 1
                    for nb in range(NH):
                        ns = slice(nb * HB, (nb + 1) * HB)
                        nc.tensor.matmul(
                            ps[:, ns], lhs, x_res[:, ct, ns],
                            start=first, stop=last,
                        )

                # epilogue: y = psum + bias (per-Cout col), then store
                t1 = p_out.tile([128, n_shard], F32, tag="t1")
                nc.vector.tensor_scalar_add(t1, ps, ball[:, og:og + 1])
                nc.scalar.dma_start(yt[og * 128:(og + 1) * 128, :], t1)

    nc.finalize()
    return nc


def _prep_weights(w, n_og, n_ct):
    # [cout, cin] -> [n_og, 128(p=cin sub), n_ct*128(o)] where
    # out[og, p, ct*128+o] = w[og*128+o, ct*128+p]
    cout, cin = w.shape
    wr = w.reshape(n_og, 128, n_ct, 128)        # [og, o, ct, p]
    wr = wr.transpose(0, 3, 2, 1)               # [og, p, ct, o]
    return np.ascontiguousarray(wr.reshape(n_og, 128, n_ct * 128)).astype(
        ml_dtypes.bfloat16
    )


def kernel(x, w0, w1, s0, s1, bias):
    from concourse.bass_utils import run_bass_kernel_spmd

    N, Cin = x.shape[0], x.shape[1]
    Cout = w0.shape[0]
    n_shard = N // NCORES
    n_ct = Cin // 128
    n_og = Cout // 128

    x2d = np.asarray(x, dtype=np.float32).reshape(N, Cin)
    Wc = (
        np.asarray(s0, np.float32)[:, None] * np.asarray(w0, np.float32)
        + np.asarray(s1, np.float32)[:, None] * np.asarray(w1, np.float32)
    )
    wch = _prep_weights(Wc, n_og, n_ct)
    bpp = np.ascontiguousarray(
        np.asarray(bias, np.float32).reshape(n_og, 128).T
    )

    nc = build_nc(n_shard=n_shard, cin=Cin, cout=Cout)

    in_maps = []
    for k in range(NCORES):
        # xT [cin, n_shard] -> [n_ct/2, 128, 2, n_shard] ct-pair chunks
        xtk = np.ascontiguousarray(
            x2d[k * n_shard:(k + 1) * n_shard].T
            .reshape(n_ct // 2, 2, 128, n_shard)
            .transpose(0, 2, 1, 3)
        ).astype(ml_dtypes.bfloat16)
        in_maps.append({"xtb": xtk, "wch": wch, "bp": bpp})

    res = run_bass_kernel_spmd(
        nc,
        in_maps,
        core_ids=list(range(NCORES)),
        trace=bool(int(os.environ.get("KERNEL_TRACE", "0"))),
    )

    y = np.empty((N, Cout), dtype=np.float32)
    for k in range(NCORES):
        y[k * n_shard:(k + 1) * n_shard] = res.results[k]["yt"].T
    out = y.reshape(N, Cout, 1, 1)
    kernel.last_results = res
    return out


# revision 30
# speedup vs baseline: 1.0113x; 1.0063x over previous
"""Trainium2 Bass kernel for nn_DualInt8LinearConv.

Reference computation (N=8192, Cin=4096, Cout=4096):
    x2d      = x.reshape(N, Cin)
    amax     = max(|x2d|, axis=1)
    scale_x  = softplus(amax) / 32767
    xq       = round(x2d / scale_x)                    # int16-valued
    y        = (xq@w0.T * s0 + xq@w1.T * s1) * scale_x + bias

Key identity: s0/s1 are per-Cout scalars, so
    xq@w0.T * s0 + xq@w1.T * s1  ==  xq @ (s0[:,None]*w0 + s1[:,None]*w1).T
and (xq * scale_x) == x up to the int16 fake-quant rounding (|eps| <=
scale_x/2 ~ 6e-5, never clipped since softplus(amax) > amax). Hence

    y  ~=  x @ Wc.T + bias,   Wc = s0[:,None]*w0 + s1[:,None]*w1

exactly up to quantization noise. Computing this single GEMM with x and
Wc rounded to bf16 (fp32 PSUM accumulation) gives max scale-relative
error ~2.4e-3 on the reference data — an order of magnitude inside the
2e-2 gate — while doing 1/4 of the matmul work of the exact dual-plane
int16xint8 decomposition.

Strategy:
  * Row-shard N across 8 cores (1024 rows each); Wc replicated.
  * Host prep: Wc combined + cast bf16 + pre-tiled [n_og, 128, n_ct*128]
    so each 128-Cout group streams as contiguous DMAs; x transposed to
    xT [Cin, n_shard] bf16 (contraction on partitions, no on-device
    transposes anywhere).
  * Device: xT resident in SBUF (8 MB). For each Cout group og:
    stream w tile, accumulate psum[128, n_shard] over 32 Cin tiles,
    epilogue = single vector op (psum + bias -> SBUF), DMA out yT.
  * Output yT [Cout, n_shard] f32; host transposes back.
"""

import os
import sys

sys.path.insert(0, "/opt/trn_rl_repo")

from contextlib import ExitStack

import numpy as np
import ml_dtypes

import concourse.bass as bass
import concourse.mybir as mybir
from concourse import bacc
import concourse.tile as tile

F32 = mybir.dt.float32
BF16 = mybir.dt.bfloat16
AF = mybir.ActivationFunctionType
ALU = mybir.AluOpType

N_FULL, CIN, COUT = 8192, 4096, 4096
NCORES = 8


def build_nc(n_shard=N_FULL // NCORES, cin=CIN, cout=COUT):
    n_ct = cin // 128       # Cin tiles (contraction)
    n_og = cout // 128      # Cout groups
    HB = min(512, n_shard)  # matmul moving width (one PSUM bank, f32)
    NH = n_shard // HB
    QS = min(8, n_ct)       # cts per weight quarter-tile
    NQ = n_ct // QS         # weight DMAs per og

    nc = bacc.Bacc()
    # x host-tiled as [n_ct/2, 128, 2, n_shard]: chunk k holds ct pair
    # (2k, 2k+1) partition-major, so each 512KB chunk DMA is fully
    # contiguous in DRAM and lands partition-contiguous in SBUF.
    xtb = nc.declare_dram_parameter(
        "xtb", [n_ct // 2, 128, 2, n_shard], BF16, isOutput=False
    )
    wch = nc.declare_dram_parameter("wch", [n_og, 128, n_ct * 128], BF16, isOutput=False)
    bp = nc.declare_dram_parameter("bp", [128, n_og], F32, isOutput=False)
    yt = nc.declare_dram_parameter("yt", [cout, n_shard], BF16, isOutput=True)

    with tile.TileContext(nc) as tc, ExitStack() as ctx:
        p_x = ctx.enter_context(tc.tile_pool(name="xres", bufs=1))
        p_w = ctx.enter_context(tc.tile_pool(name="wts", bufs=8))
        p_out = ctx.enter_context(tc.tile_pool(name="out", bufs=4))
        p_s = ctx.enter_context(tc.tile_pool(name="svec", bufs=1))

        ball = p_s.tile([128, n_og], F32)
        nc.sync.dma_start(ball, bp[:])

        # xT resident in SBUF (one big tile; slice-level deps let og0
        # consume ct tiles as they land; gpsimd queue, one engine =>
        # one monotone semaphore for all x waits). 2-ct chunks: issue
        # pace 0.65us/2ct beats og0's 0.88us/2ct consumption, and the
        # first 512KB chunk lands ~1.4us after issue.
        x_res = p_x.tile([128, n_ct, n_shard], BF16)
        for k in range(n_ct // 2):
            nc.gpsimd.dma_start(x_res[:, 2 * k:2 * k + 2, :], xtb[k])

        with tc.tile_pool(name="ps", bufs=4, space="PSUM") as p_ps:
            for og in range(n_og):
                # stream this og's weights: NQ quarter tiles (sync queue)
                wq = []
                for q in range(NQ):
                    wt_ = p_w.tile([128, QS * 128], BF16, tag="w")
                    nc.sync.dma_start(
                        wt_, wch[og, :, q * QS * 128:(q + 1) * QS * 128]
                    )
                    wq.append(wt_)

                ps = p_ps.tile([128, n_shard], F32, tag="ps")
                for ct in range(n_ct):
                    lhs = wq[ct // QS][:, (ct % QS) * 128:(ct % QS + 1) * 128]
                    first, last = ct == 0, ct == n_ct - 1
                    for nb in range(NH):
                        ns = slice(nb * HB, (nb + 1) * HB)
                        nc.tensor.matmul(
                            ps[:, ns], lhs, x_res[:, ct, ns],
                            start=first, stop=last,
                        )

                # epilogue: y = psum + bias (per-Cout col), then store
                # as bf16 (half the store bytes; host upcasts). Last og
                # splits in halves so its store overlaps its add.
                t1 = p_out.tile([128, n_shard], BF16, tag="t1")
                if og == n_og - 1:
                    for nb in range(NH):
                        ns = slice(nb * HB, (nb + 1) * HB)
                        nc.vector.tensor_scalar_add(
                            t1[:, ns], ps[:, ns], ball[:, og:og + 1]
                        )
                        nc.scalar.dma_start(
                            yt[og * 128:(og + 1) * 128, ns], t1[:, ns]
                        )
                else:
                    nc.vector.tensor_scalar_add(t1, ps, ball[:, og:og + 1])
                    nc.scalar.dma_start(yt[og * 128:(og + 1) * 128, :], t1)

    nc.finalize()
    return nc


def _prep_weights(w, n_og, n_ct):
    # [cout, cin] -> [n_og, 128(p=cin sub), n_ct*128(o)] where
    # out[og, p, ct*128+o] = w[og*128+o, ct*128+p]
    cout, cin = w.shape
    wr = w.reshape(n_og, 128, n_ct, 128)        # [og, o, ct, p]
    wr = wr.transpose(0, 3, 2, 1)               # [og, p, ct, o]
    return np.ascontiguousarray(wr.reshape(n_og, 128, n_ct * 128)).astype(
        ml_dtypes.bfloat16
    )


def kernel(x, w0, w1, s0, s1, bias):
    from concourse.bass_utils import run_bass_kernel_spmd

    N, Cin = x.shape[0], x.shape[1]
    Cout = w0.shape[0]
    n_shard = N // NCORES
    n_ct = Cin // 128
    n_og = Cout // 128

    x2d = np.asarray(x, dtype=np.float32).reshape(N, Cin)
    Wc = (
        np.asarray(s0, np.float32)[:, None] * np.asarray(w0, np.float32)
        + np.asarray(s1, np.float32)[:, None] * np.asarray(w1, np.float32)
    )
    wch = _prep_weights(Wc, n_og, n_ct)
    bpp = np.ascontiguousarray(
        np.asarray(bias, np.float32).reshape(n_og, 128).T
    )

    nc = build_nc(n_shard=n_shard, cin=Cin, cout=Cout)

    in_maps = []
    for k in range(NCORES):
        # xT [cin, n_shard] -> [n_ct/2, 128, 2, n_shard] ct-pair chunks
        xtk = np.ascontiguousarray(
            x2d[k * n_shard:(k + 1) * n_shard].T
            .reshape(n_ct // 2, 2, 128, n_shard)
            .transpose(0, 2, 1, 3)
        ).astype(ml_dtypes.bfloat16)
        in_maps.append({"xtb": xtk, "wch": wch, "bp": bpp})

    res = run_bass_kernel_spmd(
        nc,
        in_maps,
        core_ids=list(range(NCORES)),
        trace=bool(int(os.environ.get("KERNEL_TRACE", "0"))),
    )

    y = np.empty((N, Cout), dtype=np.float32)
    for k in range(NCORES):
        y[k * n_shard:(k + 1) * n_shard] = (
            res.results[k]["yt"].astype(np.float32).T
        )
    out = y.reshape(N, Cout, 1, 1)
    kernel.last_results = res
    return out
